# revision 2
# baseline (speedup 1.0000x reference)
"""TRN2 Bass kernel for nn_Decoder_24146306138719.

Math: reference computes, for the top-8 (by slogdet) of 16 matrices
[T0..T3 = interact, P0..P3 = preserve]:
    pairs[p]   = T_i1p @ T_i2p  for the 6 upper-tri pairs of T0..T3
    concat     = [pairs(6), P0..P3]                    (10, n, n)
    out        = w2 @ (w1 @ concat + b1) + b2          1x1 convs
Everything after the pair matmuls is linear, so with
    weff = w2 @ w1  (10,),  beff = w2 @ b1 + b2  (scalar)
    OUT = sum_p weff[p] * pairs[p] + sum_q weff[6+q] * P_q + beff
Grouping the 6 weighted pair products by left factor:
    OUT = T0 @ R0 + T1 @ R1 + T2 @ R2 + S
    R0 = w0*T1 + w1*T2 + w2*T3; R1 = w3*T2 + w4*T3; R2 = w5*T3
    S  = c6*P0 + c7*P1 + c8*P2 + c9*P3 + beff
i.e. ONE K=3072 matmul (6.4 GF instead of 12.9 GF) plus elementwise terms.
On device we fold the weights into the stationary (lhsT) side instead:
    OUT = L1 @ T1 + L2 @ T2 + L3 @ T3 + S
    L1 = w0*T0; L2 = w1*T0 + w3*T1; L3 = w2*T0 + w4*T1 + w5*T2
so the streaming operands are raw T slices and the small lhsT tiles get
the vector-engine combination work.

Sharding: output (1024,1024) split 4 (rows: M_s=256) x 2 (cols: N_s=512)
across the 8 cores.  slogdet ranking + slicing/transposes on host; all
matmul + elementwise compute on device.
"""

import numpy as np

N = 1024
P = 128
MS = 256  # per-core output rows   (4 row groups)
NS = 512  # per-core output cols   (2 col groups)
KT = 8  # k-tiles per 1024 contraction
THRESH = 4
TOP_K = 8
INTERACT = 4

_CACHE = {}


def _build_bass():
    from concourse import bacc
    import concourse.mybir as mybir
    import concourse.tile as tile

    f32 = mybir.dt.float32
    nc = bacc.Bacc(None)

    lhsT = nc.declare_dram_parameter("lhsT", [3, P, KT, MS], f32, isOutput=False)
    rin = nc.declare_dram_parameter("rin", [3, P, KT, NS], f32, isOutput=False)
    pres = nc.declare_dram_parameter("pres", [4, P, 2, NS], f32, isOutput=False)
    cw = nc.declare_dram_parameter("cw", [P, 12], f32, isOutput=False)
    out = nc.declare_dram_parameter("out", [2, P, NS], f32, isOutput=True)

    mult = mybir.AluOpType.mult
    add = mybir.AluOpType.add

    with tile.TileContext(nc) as tc:
        with (
            tc.tile_pool(name="const", bufs=1) as const,
            tc.tile_pool(name="sb_l", bufs=1) as sb_l,
            tc.tile_pool(name="sb_r", bufs=1) as sb_r,
            tc.tile_pool(name="sb_p", bufs=1) as sb_p,
            tc.tile_pool(name="sb_lt", bufs=1) as sb_lt,
            tc.tile_pool(name="sb_s", bufs=1) as sb_s,
            tc.tile_pool(name="ps", bufs=2, space="PSUM") as ps,
            tc.tile_pool(name="sb_o", bufs=2) as sb_o,
        ):
            sb_cw = const.tile([P, 12], f32)
            nc.sync.dma_start(out=sb_cw[:], in_=cw[:])

            t_l = [sb_l.tile([P, KT, MS], f32, tag=f"l{j}", name=f"l{j}") for j in range(3)]
            t_r = [sb_r.tile([P, KT, NS], f32, tag=f"r{l}", name=f"r{l}") for l in range(3)]
            t_p = [sb_p.tile([P, 2, NS], f32, tag=f"p{q}", name=f"p{q}") for q in range(4)]
            for j in range(3):
                nc.sync.dma_start(out=t_l[j][:], in_=lhsT[j])
            for l in range(3):
                nc.sync.dma_start(out=t_r[l][:], in_=rin[l])
            for q in range(4):
                nc.sync.dma_start(out=t_p[q][:], in_=pres[q])

            def w(i):
                return sb_cw[:, i : i + 1]

            # Lt[l] tiles ([P, KT, MS]) = transposed stationary operands.
            t_lt = [sb_lt.tile([P, KT, MS], f32, tag=f"lt{l}", name=f"lt{l}") for l in range(3)]
            # L1 = w0*T0
            nc.scalar.mul(t_lt[0][:], t_l[0][:], w(0))
            # L2 = w1*T0 + w3*T1
            nc.scalar.mul(t_lt[1][:], t_l[0][:], w(1))
            nc.vector.scalar_tensor_tensor(
                t_lt[1][:], t_l[1][:], w(3), t_lt[1][:], mult, add
            )
            # L3 = w2*T0 + w4*T1 + w5*T2
            nc.scalar.mul(t_lt[2][:], t_l[0][:], w(2))
            nc.vector.scalar_tensor_tensor(
                t_lt[2][:], t_l[1][:], w(4), t_lt[2][:], mult, add
            )
            nc.vector.scalar_tensor_tensor(
                t_lt[2][:], t_l[2][:], w(5), t_lt[2][:], mult, add
            )

            # S[mm] = c6*P0 + beff + c7*P1 + c8*P2 + c9*P3
            t_s = [sb_s.tile([P, NS], f32, tag=f"s{mm}", name=f"s{mm}") for mm in range(2)]
            for mm in range(2):
                nc.vector.tensor_scalar(
                    t_s[mm][:], t_p[0][:, mm, :], w(6), w(10), mult, add
                )
                for q in range(1, 4):
                    nc.vector.scalar_tensor_tensor(
                        t_s[mm][:], t_p[q][:, mm, :], w(6 + q), t_s[mm][:], mult, add
                    )

            for mm in range(2):
                acc = ps.tile([P, NS], f32, tag=f"acc{mm}", name=f"acc{mm}")
                for l in range(3):
                    for kk in range(KT):
                        nc.tensor.matmul(
                            acc[:],
                            t_lt[l][:, kk, mm * P : (mm + 1) * P],
                            t_r[l][:, kk, :],
                            start=(l == 0 and kk == 0),
                            stop=(l == 2 and kk == KT - 1),
                        )
                o = sb_o.tile([P, NS], f32, tag=f"o{mm}", name=f"o{mm}")
                nc.vector.tensor_add(o[:], acc[:], t_s[mm][:])
                nc.sync.dma_start(out=out[mm], in_=o[:])

    nc.compile()
    return nc


def _get_runner():
    """Build + jit once per process; returns fn(in_maps) -> list of out dicts."""
    if "runner" in _CACHE:
        return _CACHE["runner"]

    import jax
    import numpy as _np
    from jax.sharding import Mesh, PartitionSpec
    from jax.experimental.shard_map import shard_map
    import concourse.mybir as mybir
    from concourse import bass2jax
    from concourse.bass2jax import _bass_exec_p, partition_id_tensor

    nc = _build_bass()
    bass2jax.install_neuronx_cc_hook()

    in_names, out_names, out_avals, zero_shapes = [], [], [], []
    partition_name = nc.partition_id_tensor.name if nc.partition_id_tensor else None
    for alloc in nc.m.functions[0].allocations:
        if not isinstance(alloc, mybir.MemoryLocationSet):
            continue
        name = alloc.memorylocations[0].name
        if alloc.kind == "ExternalInput":
            if name != partition_name:
                in_names.append(name)
        elif alloc.kind == "ExternalOutput":
            out_names.append(name)
            shape = tuple(alloc.tensor_shape)
            dtype = mybir.dt.np(alloc.dtype)
            out_avals.append(jax.core.ShapedArray(shape, dtype))
            zero_shapes.append((shape, dtype))
    n_params = len(in_names)
    n_outs = len(out_avals)
    all_in_names = list(in_names) + list(out_names)
    if partition_name is not None:
        all_in_names.append(partition_name)
    donate = tuple(range(n_params, n_params + n_outs))

    def _body(*args):
        operands = list(args)
        if partition_name is not None:
            operands.append(partition_id_tensor())
        outs = _bass_exec_p.bind(
            *operands,
            out_avals=tuple(out_avals),
            in_names=tuple(all_in_names),
            out_names=tuple(out_names),
            lowering_input_output_aliases=(),
            sim_require_finite=True,
            sim_require_nnan=True,
            nc=nc,
        )
        return tuple(outs)

    n_cores = 8
    devices = jax.devices()[:n_cores]
    mesh = Mesh(_np.asarray(devices), ("core",))
    in_specs = (PartitionSpec("core"),) * (n_params + n_outs)
    out_specs = (PartitionSpec("core"),) * n_outs
    sharded = jax.jit(
        shard_map(
            _body, mesh=mesh, in_specs=in_specs, out_specs=out_specs, check_rep=False
        ),
        donate_argnums=donate,
        keep_unused=True,
    )

    def runner(in_maps):
        concat_in = [
            _np.concatenate([_np.asarray(m[name]) for m in in_maps], axis=0)
            for name in in_names
        ]
        concat_zeros = [
            _np.zeros((n_cores * s[0], *s[1:]), dt) for (s, dt) in zero_shapes
        ]
        out_arrs = sharded(*concat_in, *concat_zeros)
        return [
            {
                name: _np.asarray(out_arrs[i]).reshape(n_cores, *out_avals[i].shape)[c]
                for i, name in enumerate(out_names)
            }
            for c in range(n_cores)
        ]

    _CACHE["runner"] = runner
    return runner


def _select_indices(x, is_active_flags):
    """Replicate slogdet ranking + lax.top_k tie-break (lowest index first)."""
    _, logabs = np.linalg.slogdet(x.astype(np.float64))
    scores = logabs.astype(np.float32)
    scores = np.where(np.asarray(is_active_flags, dtype=bool), scores, -np.inf)
    order = np.argsort(-scores, kind="stable")[:TOP_K]
    return order.astype(np.int64)


def kernel(x, is_active_flags, w1, b1, w2, b2):
    x = np.asarray(x, dtype=np.float32)
    flags = np.asarray(is_active_flags, dtype=bool)
    w1 = np.asarray(w1, dtype=np.float32)
    b1 = np.asarray(b1, dtype=np.float32)
    w2 = np.asarray(w2, dtype=np.float32)
    b2 = np.asarray(b2, dtype=np.float32)

    n = x.shape[1]
    if int(flags.sum()) < THRESH:
        return (
            np.zeros((1, n, n), dtype=np.float32),
            np.array(False),
        )

    idx = _select_indices(x, flags)
    T = [x[idx[j]] for j in range(INTERACT)]
    Pm = [x[idx[INTERACT + q]] for q in range(4)]

    weff = (w2 @ w1).reshape(-1)  # (10,)
    beff = float((w2 @ b1.reshape(w1.shape[0], 1)).reshape(()) + b2.reshape(()))

    Tt = [np.ascontiguousarray(T[j].T) for j in range(3)]  # [k, m]

    cw = np.zeros((P, 12), dtype=np.float32)
    cw[:, :10] = weff[np.newaxis, :]
    cw[:, 10] = beff

    in_maps = []
    for c in range(8):
        mg, ng = divmod(c, 2)
        ms, ns = mg * MS, ng * NS
        lhsT = np.stack(
            [
                Tt[j][:, ms : ms + MS].reshape(KT, P, MS).transpose(1, 0, 2)
                for j in range(3)
            ]
        )
        rin = np.stack(
            [
                T[l][:, ns : ns + NS].reshape(KT, P, NS).transpose(1, 0, 2)
                for l in range(1, 4)
            ]
        )
        pres = np.stack(
            [
                Pm[q][ms : ms + MS, ns : ns + NS].reshape(2, P, NS).transpose(1, 0, 2)
                for q in range(4)
            ]
        )
        in_maps.append(
            {
                "lhsT": np.ascontiguousarray(lhsT),
                "rin": np.ascontiguousarray(rin),
                "pres": np.ascontiguousarray(pres),
                "cw": cw,
            }
        )

    runner = _get_runner()
    results = runner(in_maps)

    OUT = np.empty((n, n), dtype=np.float32)
    for c in range(8):
        mg, ng = divmod(c, 2)
        o = results[c]["out"]  # [2, P, NS]
        for mm in range(2):
            OUT[mg * MS + mm * P : mg * MS + (mm + 1) * P, ng * NS : (ng + 1) * NS] = o[
                mm
            ]
    return (OUT[np.newaxis], np.array(True))


# revision 3
# speedup vs baseline: 1.3822x; 1.3822x over previous
"""TRN2 Bass kernel for nn_Decoder_24146306138719.

Math: reference computes, for the top-8 (by slogdet) of 16 matrices
[T0..T3 = interact, P0..P3 = preserve]:
    pairs[p]   = T_i1p @ T_i2p  for the 6 upper-tri pairs of T0..T3
    concat     = [pairs(6), P0..P3]                    (10, n, n)
    out        = w2 @ (w1 @ concat + b1) + b2          1x1 convs
Everything after the pair matmuls is linear, so with
    weff = w2 @ w1  (10,),  beff = w2 @ b1 + b2  (scalar)
    OUT = sum_p weff[p] * pairs[p] + sum_q weff[6+q] * P_q + beff
Grouping the 6 weighted pair products by RIGHT factor:
    OUT = L1 @ T1 + L2 @ T2 + L3 @ T3 + S
    L1 = w0*T0; L2 = w1*T0 + w3*T1; L3 = w2*T0 + w4*T1 + w5*T2
    S  = c6*P0 + c7*P1 + c8*P2 + c9*P3 + beff
i.e. ONE K=3072 contraction (6.4 GF instead of 12.9 GF) plus cheap
elementwise terms.

Precision: matmuls run as 3-pass bf16 hi/lo split (C = Ah@Bh + Ah@Bl +
Al@Bh, fp32 PSUM accumulate) - measured ~5e-6 rel err, 3x faster than
TRN2's quarter-rate native fp32 matmul.

Sharding: output (1024,1024) split 4 (rows: M_s=256) x 2 (cols: N_s=512)
across 8 cores.  slogdet ranking, slicing, transposes, weight folding and
the bf16 split happen on host; all O(n^3) matmul work + the preserve/bias
elementwise term run on device.
"""

import numpy as np

N = 1024
P = 128
MS = 256  # per-core output rows   (4 row groups)
NS = 512  # per-core output cols   (2 col groups)
KT = 8  # k-tiles per 1024 contraction
THRESH = 4
TOP_K = 8
INTERACT = 4

_CACHE = {}


def _build_bass():
    from concourse import bacc
    import concourse.mybir as mybir
    import concourse.tile as tile

    f32 = mybir.dt.float32
    bf16 = mybir.dt.bfloat16
    nc = bacc.Bacc(None)

    # hi/lo bf16 stationary (pre-transposed, pre-combined L1..L3) and
    # hi/lo bf16 moving (raw T1..T3 column slices)
    lthi = nc.declare_dram_parameter("lthi", [3, P, KT, MS], bf16, isOutput=False)
    ltlo = nc.declare_dram_parameter("ltlo", [3, P, KT, MS], bf16, isOutput=False)
    rhi = nc.declare_dram_parameter("rhi", [3, P, KT, NS], bf16, isOutput=False)
    rlo = nc.declare_dram_parameter("rlo", [3, P, KT, NS], bf16, isOutput=False)
    pres = nc.declare_dram_parameter("pres", [4, P, 2, NS], f32, isOutput=False)
    cw = nc.declare_dram_parameter("cw", [P, 12], f32, isOutput=False)
    out = nc.declare_dram_parameter("out", [2, P, NS], f32, isOutput=True)

    mult = mybir.AluOpType.mult
    add = mybir.AluOpType.add

    with tile.TileContext(nc) as tc:
        with (
            tc.tile_pool(name="const", bufs=1) as const,
            tc.tile_pool(name="sb_l", bufs=1) as sb_l,
            tc.tile_pool(name="sb_r", bufs=1) as sb_r,
            tc.tile_pool(name="sb_p", bufs=1) as sb_p,
            tc.tile_pool(name="sb_s", bufs=1) as sb_s,
            tc.tile_pool(name="ps", bufs=2, space="PSUM") as ps,
            tc.tile_pool(name="sb_o", bufs=2) as sb_o,
        ):
            sb_cw = const.tile([P, 12], f32)
            nc.sync.dma_start(out=sb_cw[:], in_=cw[:])

            t_lh = [sb_l.tile([P, KT, MS], bf16, tag=f"lh{l}", name=f"lh{l}") for l in range(3)]
            t_ll = [sb_l.tile([P, KT, MS], bf16, tag=f"ll{l}", name=f"ll{l}") for l in range(3)]
            t_rh = [sb_r.tile([P, KT, NS], bf16, tag=f"rh{l}", name=f"rh{l}") for l in range(3)]
            t_rl = [sb_r.tile([P, KT, NS], bf16, tag=f"rl{l}", name=f"rl{l}") for l in range(3)]
            t_p = [sb_p.tile([P, 2, NS], f32, tag=f"p{q}", name=f"p{q}") for q in range(4)]

            # DMA order = consumption order: hi tiles first (hh pass),
            # then lo tiles, then preserve.  Halves of each r tile so the
            # first matmuls can start early.
            for l in range(3):
                nc.sync.dma_start(out=t_lh[l][:], in_=lthi[l])
                nc.sync.dma_start(out=t_rh[l][:, 0:4, :], in_=rhi[l, :, 0:4, :])
                nc.sync.dma_start(out=t_rh[l][:, 4:8, :], in_=rhi[l, :, 4:8, :])
            for l in range(3):
                nc.sync.dma_start(out=t_ll[l][:], in_=ltlo[l])
                nc.sync.dma_start(out=t_rl[l][:, 0:4, :], in_=rlo[l, :, 0:4, :])
                nc.sync.dma_start(out=t_rl[l][:, 4:8, :], in_=rlo[l, :, 4:8, :])
            for q in range(4):
                nc.sync.dma_start(out=t_p[q][:], in_=pres[q])

            def w(i):
                return sb_cw[:, i : i + 1]

            # S[mm] = c6*P0 + beff + c7*P1 + c8*P2 + c9*P3
            t_s = [sb_s.tile([P, NS], f32, tag=f"s{mm}", name=f"s{mm}") for mm in range(2)]
            for mm in range(2):
                nc.vector.tensor_scalar(
                    t_s[mm][:], t_p[0][:, mm, :], w(6), w(10), mult, add
                )
                for q in range(1, 4):
                    nc.vector.scalar_tensor_tensor(
                        t_s[mm][:], t_p[q][:, mm, :], w(6 + q), t_s[mm][:], mult, add
                    )

            for mm in range(2):
                acc = ps.tile([P, NS], f32, tag=f"acc{mm}", name=f"acc{mm}")
                passes = [(t_lh, t_rh), (t_lh, t_rl), (t_ll, t_rh)]
                first = True
                for pi, (tl, tr) in enumerate(passes):
                    for l in range(3):
                        for kk in range(KT):
                            nc.tensor.matmul(
                                acc[:],
                                tl[l][:, kk, mm * P : (mm + 1) * P],
                                tr[l][:, kk, :],
                                start=first,
                                stop=(pi == 2 and l == 2 and kk == KT - 1),
                            )
                            first = False
                o = sb_o.tile([P, NS], f32, tag=f"o{mm}", name=f"o{mm}")
                nc.vector.tensor_add(o[:], acc[:], t_s[mm][:])
                nc.sync.dma_start(out=out[mm], in_=o[:])

    nc.compile()
    return nc


def _get_runner():
    """Build + jit once per process; returns fn(in_maps) -> list of out dicts."""
    if "runner" in _CACHE:
        return _CACHE["runner"]

    import jax
    import numpy as _np
    from jax.sharding import Mesh, PartitionSpec
    from jax.experimental.shard_map import shard_map
    import concourse.mybir as mybir
    from concourse import bass2jax
    from concourse.bass2jax import _bass_exec_p, partition_id_tensor

    nc = _build_bass()
    bass2jax.install_neuronx_cc_hook()

    in_names, out_names, out_avals, zero_shapes = [], [], [], []
    partition_name = nc.partition_id_tensor.name if nc.partition_id_tensor else None
    for alloc in nc.m.functions[0].allocations:
        if not isinstance(alloc, mybir.MemoryLocationSet):
            continue
        name = alloc.memorylocations[0].name
        if alloc.kind == "ExternalInput":
            if name != partition_name:
                in_names.append(name)
        elif alloc.kind == "ExternalOutput":
            out_names.append(name)
            shape = tuple(alloc.tensor_shape)
            dtype = mybir.dt.np(alloc.dtype)
            out_avals.append(jax.core.ShapedArray(shape, dtype))
            zero_shapes.append((shape, dtype))
    n_params = len(in_names)
    n_outs = len(out_avals)
    all_in_names = list(in_names) + list(out_names)
    if partition_name is not None:
        all_in_names.append(partition_name)
    donate = tuple(range(n_params, n_params + n_outs))

    def _body(*args):
        operands = list(args)
        if partition_name is not None:
            operands.append(partition_id_tensor())
        outs = _bass_exec_p.bind(
            *operands,
            out_avals=tuple(out_avals),
            in_names=tuple(all_in_names),
            out_names=tuple(out_names),
            lowering_input_output_aliases=(),
            sim_require_finite=True,
            sim_require_nnan=True,
            nc=nc,
        )
        return tuple(outs)

    n_cores = 8
    devices = jax.devices()[:n_cores]
    mesh = Mesh(_np.asarray(devices), ("core",))
    in_specs = (PartitionSpec("core"),) * (n_params + n_outs)
    out_specs = (PartitionSpec("core"),) * n_outs
    sharded = jax.jit(
        shard_map(
            _body, mesh=mesh, in_specs=in_specs, out_specs=out_specs, check_rep=False
        ),
        donate_argnums=donate,
        keep_unused=True,
    )

    def runner(in_maps):
        concat_in = [
            _np.concatenate([_np.asarray(m[name]) for m in in_maps], axis=0)
            for name in in_names
        ]
        concat_zeros = [
            _np.zeros((n_cores * s[0], *s[1:]), dt) for (s, dt) in zero_shapes
        ]
        out_arrs = sharded(*concat_in, *concat_zeros)
        return [
            {
                name: _np.asarray(out_arrs[i]).reshape(n_cores, *out_avals[i].shape)[c]
                for i, name in enumerate(out_names)
            }
            for c in range(n_cores)
        ]

    _CACHE["runner"] = runner
    return runner


def _select_indices(x, is_active_flags):
    """Replicate slogdet ranking + lax.top_k tie-break (lowest index first)."""
    _, logabs = np.linalg.slogdet(x)
    scores = logabs.astype(np.float32)
    scores = np.where(np.asarray(is_active_flags, dtype=bool), scores, -np.inf)
    order = np.argsort(-scores, kind="stable")[:TOP_K]
    return order.astype(np.int64)


def _split_bf16(a):
    import ml_dtypes

    hi = a.astype(ml_dtypes.bfloat16)
    lo = (a - hi.astype(np.float32)).astype(ml_dtypes.bfloat16)
    return hi, lo


def build_in_maps(x, flags, w1, b1, w2, b2):
    """Host prep: select, fold weights, combine L's, transpose, split, slice."""
    idx = _select_indices(x, flags)
    T = [x[idx[j]] for j in range(INTERACT)]
    Pm = [x[idx[INTERACT + q]] for q in range(4)]

    weff = (w2 @ w1).reshape(-1)  # (10,)
    beff = float((w2 @ b1.reshape(w1.shape[0], 1)).reshape(()) + b2.reshape(()))

    # Transposed, weight-combined stationary operands (f32), then bf16 split.
    Tt = [np.ascontiguousarray(T[j].T) for j in range(3)]  # [k, m]
    Lt = [
        weff[0] * Tt[0],
        weff[1] * Tt[0] + weff[3] * Tt[1],
        weff[2] * Tt[0] + weff[4] * Tt[1] + weff[5] * Tt[2],
    ]
    Lt_hi, Lt_lo = zip(*[_split_bf16(L) for L in Lt])
    R_hi, R_lo = zip(*[_split_bf16(T[l]) for l in range(1, 4)])

    cw = np.zeros((P, 12), dtype=np.float32)
    cw[:, :10] = weff[np.newaxis, :]
    cw[:, 10] = beff

    def ltile(arrs, ms):
        return np.ascontiguousarray(
            np.stack(
                [a[:, ms : ms + MS].reshape(KT, P, MS).transpose(1, 0, 2) for a in arrs]
            )
        )

    def rtile(arrs, ns):
        return np.ascontiguousarray(
            np.stack(
                [a[:, ns : ns + NS].reshape(KT, P, NS).transpose(1, 0, 2) for a in arrs]
            )
        )

    in_maps = []
    for c in range(8):
        mg, ng = divmod(c, 2)
        ms, ns = mg * MS, ng * NS
        pres = np.ascontiguousarray(
            np.stack(
                [
                    Pm[q][ms : ms + MS, ns : ns + NS]
                    .reshape(2, P, NS)
                    .transpose(1, 0, 2)
                    for q in range(4)
                ]
            )
        )
        in_maps.append(
            {
                "lthi": ltile(Lt_hi, ms),
                "ltlo": ltile(Lt_lo, ms),
                "rhi": rtile(R_hi, ns),
                "rlo": rtile(R_lo, ns),
                "pres": pres,
                "cw": cw,
            }
        )
    return in_maps


def assemble(results, n=N):
    OUT = np.empty((n, n), dtype=np.float32)
    for c in range(8):
        mg, ng = divmod(c, 2)
        o = results[c]["out"]  # [2, P, NS]
        for mm in range(2):
            OUT[mg * MS + mm * P : mg * MS + (mm + 1) * P, ng * NS : (ng + 1) * NS] = o[
                mm
            ]
    return OUT


def kernel(x, is_active_flags, w1, b1, w2, b2):
    x = np.asarray(x, dtype=np.float32)
    flags = np.asarray(is_active_flags, dtype=bool)
    w1 = np.asarray(w1, dtype=np.float32)
    b1 = np.asarray(b1, dtype=np.float32)
    w2 = np.asarray(w2, dtype=np.float32)
    b2 = np.asarray(b2, dtype=np.float32)

    n = x.shape[1]
    if int(flags.sum()) < THRESH:
        return (np.zeros((1, n, n), dtype=np.float32), np.array(False))

    in_maps = build_in_maps(x, flags, w1, b1, w2, b2)
    runner = _get_runner()
    results = runner(in_maps)
    return (assemble(results, n)[np.newaxis], np.array(True))
